# revision 16
# baseline (speedup 1.0000x reference)
"""Causal GQA attention with RoPE for Trainium2, sharded over 8 NeuronCores.

Problem: x[4,1024,2048] @ wq/wk/wv -> RoPE -> causal GQA attention -> @ wo.
H=32 q-heads, KVH=8 kv-heads (GQA rep 4), D=64.

Sharding: core = 2*b + g  (b = batch 0..3, g = head-group 0..1).
Each core handles one batch and 16 q-heads / 4 kv-heads, computing a partial
output projection; the host sums the two head-group partials per batch.

v2 design notes (all timings per the TRN2 cost model):
  - every matmul operand is bfloat16: full PE rate at any moving width
    (fp32r drops to 1/4 rate below 256 cols), half the DMA traffic, and
    2x DVE throughput on 16-bit elementwise ops.  PSUM accumulation stays
    fp32; softmax denominators and the output stay fp32.
  - all inputs are pre-packed on the host into the exact SBUF tile layout,
    so every DMA is a plain contiguous [128, N] copy with >=512B runs
    (full DMA rate) and the arrival order can be scheduled precisely.
  - rope: the pair-swap+multiply is fused into scalar_tensor_tensor ops
    ((psum * 1.0) * cos/sin) on the Pool engine (idle otherwise, no
    SBUF-access bubble); only the final add runs on DVE.  The Activation
    engine is left almost exclusively to softmax exp, which is the
    attention-phase rate limiter.
  - softmax row-sums come from 64 ones-columns appended to v: the attn@v
    matmul then lands sum(E) replicated on psum partitions 64..127, so
    normalization is one DVE reciprocal + one Pool multiply (no gpsimd
    partition_broadcast on the critical path).
  - causal masking at 128-key granularity (36 of 64 blocks per head, the
    optimum); diagonal blocks are emitted FIRST within each accumulation
    so their extra exp+mask latency hides behind the full blocks.
  - attention for the two heads of a q-chunk is interleaved kj-step by
    kj-step, and the next q-chunk's projection chain is drip-fed between
    steps, keeping the PE stream dense while exp latency drains.
"""

import os

import numpy as np

import concourse.bacc as bacc
import concourse.bass as bass
import concourse.mybir as mybir
import concourse.tile as tile
from concourse.bass_utils import run_bass_kernel_spmd

B, S, DIM = 4, 1024, 2048
H, KVH, D = 32, 8, 64
HL = H // 2        # 16 q heads per core
KVL = KVH // 2     # 4 kv heads per core
QCOLS = HL * D     # 1024
KCOLS = KVL * D    # 256
NB = 512           # matmul moving-dim block (one PSUM bank of fp32)
P = 128
KC = DIM // P      # 16 contraction chunks

F32 = mybir.dt.float32
BF = mybir.dt.bfloat16
Exp = mybir.ActivationFunctionType.Exp
MULT = mybir.AluOpType.mult


def build_program():
    nc = bacc.Bacc()

    # host-prepacked inputs: each is already in SBUF tile layout, so DMAs
    # are contiguous [128, N] row copies at full DMA rate.
    xH = nc.dram_tensor("xH", [P, 2 * KC * NB], BF, kind="ExternalInput")
    wkH = nc.dram_tensor("wkH", [P, 2 * KC * P], BF, kind="ExternalInput")
    wqH = nc.dram_tensor("wqH", [P, 4 * KC * 256], BF, kind="ExternalInput")
    wvH = nc.dram_tensor("wvH", [P, KC * KCOLS], BF, kind="ExternalInput")
    woH = nc.dram_tensor("woH", [P, 8 * 8 * 256], BF, kind="ExternalInput")
    cosP = nc.dram_tensor("cosP", [P, S], BF, kind="ExternalInput")
    sinP = nc.dram_tensor("sinP", [P, S], BF, kind="ExternalInput")
    maskP = nc.dram_tensor("maskP", [P, P], BF, kind="ExternalInput")
    outT = nc.dram_tensor("outT", [DIM, S], F32, kind="ExternalOutput")

    with tile.TileContext(nc) as tc:
        from contextlib import ExitStack
        es = ExitStack()
        with es:
            const = es.enter_context(tc.tile_pool(name="const", bufs=1))
            xtp = es.enter_context(tc.tile_pool(name="xtp", bufs=1))
            wkp = es.enter_context(tc.tile_pool(name="wkp", bufs=1))
            wvrp = es.enter_context(tc.tile_pool(name="wvrp", bufs=1))
            wstp = es.enter_context(tc.tile_pool(name="wstp", bufs=3))
            wop = es.enter_context(tc.tile_pool(name="wop", bufs=3))
            kdupp = es.enter_context(tc.tile_pool(name="kdupp", bufs=1))
            vaugp = es.enter_context(tc.tile_pool(name="vaugp", bufs=1))
            aotp = es.enter_context(tc.tile_pool(name="aotp", bufs=1))
            qrtp = es.enter_context(tc.tile_pool(name="qrtp", bufs=3))
            spool = es.enter_context(tc.tile_pool(name="spool", bufs=3))
            epool = es.enter_context(tc.tile_pool(name="epool", bufs=4))
            rpool = es.enter_context(tc.tile_pool(name="rpool", bufs=2))
            outp = es.enter_context(tc.tile_pool(name="outp", bufs=3))
            psum_mm = es.enter_context(
                tc.tile_pool(name="psum_mm", bufs=3, space="PSUM"))
            psum_oa = es.enter_context(
                tc.tile_pool(name="psum_oa", bufs=2, space="PSUM"))
            psum_sc = es.enter_context(
                tc.tile_pool(name="psum_sc", bufs=3, space="PSUM"))

            # ---- persistent tiles ----
            cost = const.tile([P, S], BF, name="cost")
            sint = const.tile([P, S], BF, name="sint")
            maskt = const.tile([P, P], BF, name="maskt")
            kdup = [kdupp.tile([P, S], BF, name=f"kdup{i}") for i in range(KVL)]
            # v with 64 ones-columns: attn@v then produces the softmax
            # denominator replicated on psum partitions 64..127.
            vaug = [[vaugp.tile([P, D + 64], BF, name=f"vaug{kv}_{ic}")
                     for ic in range(S // P)] for kv in range(KVL)]
            aot = [aotp.tile([P, S], BF, name=f"aot{j}") for j in range(8)]

            # x tiles: xt[ib][g] holds chunks 4g..4g+3, columns ib*512..+512
            xt_half = [[xtp.tile([P, 4 * NB], BF, name=f"xt{ib}_{g}")
                        for g in range(4)] for ib in range(2)]

            def xchunk(c, ib):       # [P, NB] view of x chunk c, col block ib
                g, cc = c // 4, c % 4
                return xt_half[ib][g][:, cc * NB:(cc + 1) * NB]

            wkg = [wkp.tile([P, KC * P], BF, name=f"wkg{jk}")
                   for jk in range(2)]
            wvall = wvrp.tile([P, KC * KCOLS], BF, name="wvall")
            wvt = [wvall[:, c * KCOLS:(c + 1) * KCOLS] for c in range(KC)]

            # ---- DMA issue order (single serial queue; first-needed first)
            nc.sync.dma_start(xt_half[0][0][:], xH[:, 0:4 * NB])
            nc.sync.dma_start(wkg[0][:], wkH[:, 0:KC * P])
            nc.sync.dma_start(cost[:], cosP[:])
            nc.sync.dma_start(sint[:], sinP[:])
            for g in range(1, 4):
                nc.sync.dma_start(xt_half[0][g][:],
                                  xH[:, g * 4 * NB:(g + 1) * 4 * NB])
            nc.sync.dma_start(wkg[1][:], wkH[:, KC * P:2 * KC * P])
            base1 = 4 * NB * 4
            for g in range(4):
                nc.sync.dma_start(
                    xt_half[1][g][:],
                    xH[:, base1 + g * 4 * NB:base1 + (g + 1) * 4 * NB])
            nc.sync.dma_start(wvall[:], wvH[:])

            def load_wq_pair(pair):
                wqg = wstp.tile([P, KC * 256], BF, tag="wqpair")
                nc.sync.dma_start(
                    wqg[:], wqH[:, pair * KC * 256:(pair + 1) * KC * 256])
                return wqg

            wq_pair0 = load_wq_pair(0)
            nc.sync.dma_start(maskt[:], maskP[:])

            # ones-columns of vaug (constant, disjoint from the v writes)
            for kv in range(KVL):
                for ic in range(S // P):
                    nc.gpsimd.memset(vaug[kv][ic][:, D:D + 64], 1.0)

            # ---- helpers ----
            def rope(ps, ib, dest_ap):
                """psum [128, NB] fp32 -> roped bf16 into dest_ap.

                One Act copy psum->bf16, then the 32-partition half-swaps
                are cheap 2-byte SBUF copies split across DVE and Pool."""
                sl = slice(ib * NB, (ib + 1) * NB)
                straight = spool.tile([P, NB], BF, tag="straight")
                nc.scalar.copy(straight[:], ps[:])
                swapt = spool.tile([P, NB], BF, tag="swapt")
                nc.vector.tensor_copy(swapt[0:32, :], straight[32:64, :])
                nc.vector.tensor_copy(swapt[32:64, :], straight[0:32, :])
                nc.gpsimd.tensor_copy(swapt[64:96, :], straight[96:128, :])
                nc.gpsimd.tensor_copy(swapt[96:128, :], straight[64:96, :])
                nc.vector.tensor_mul(straight[:], straight[:], cost[:, sl])
                nc.vector.tensor_mul(swapt[:], swapt[:], sint[:, sl])
                nc.vector.tensor_add(dest_ap, straight[:], swapt[:])

            def emit_k(jk, ib):
                ps = psum_mm.tile([P, NB], F32, tag="mm")
                for c in range(KC):
                    nc.tensor.matmul(
                        ps[:], wkg[jk][:, c * P:(c + 1) * P], xchunk(c, ib),
                        start=(c == 0), stop=(c == KC - 1))
                kr = spool.tile([P, NB], BF, tag="ropek")
                rope(ps, ib, kr[:])
                sl = slice(ib * NB, (ib + 1) * NB)
                for half in range(2):     # kv head 2jk+half, duplicated
                    src = kr[64 * half:64 * half + 64, :]
                    nc.gpsimd.tensor_copy(kdup[2 * jk + half][0:64, sl], src)
                    nc.gpsimd.tensor_copy(kdup[2 * jk + half][64:128, sl], src)

            def emit_v(ic):
                ps = psum_mm.tile([P, KCOLS], F32, tag="mm")
                ib, icc = ic // 4, ic % 4
                for c in range(KC):
                    nc.tensor.matmul(
                        ps[:], xchunk(c, ib)[:, icc * P:(icc + 1) * P],
                        wvt[c][:], start=(c == 0), stop=(c == KC - 1))
                for kv in range(KVL):
                    nc.scalar.copy(vaug[kv][ic][:, 0:D],
                                   ps[:, kv * D:(kv + 1) * D])

            def proj_q_steps(jq, wqg, qr):
                """Yield per-matmul steps of the q projection for chunk jq
                (both 512-col blocks), with the rope chains emitted after
                each block completes."""
                off = (jq % 2) * P
                for ib in range(2):
                    ps = psum_mm.tile([P, NB], F32, tag="mm")
                    for c in range(KC):
                        nc.tensor.matmul(
                            ps[:], wqg[:, c * 256 + off:c * 256 + off + P],
                            xchunk(c, ib), start=(c == 0), stop=(c == KC - 1))
                        yield
                    rope(ps, ib, qr[:, ib * NB:(ib + 1) * NB])
                    yield

            def emit_attention(jq, qr, filler):
                """Attention for the two heads of q-chunk jq; `filler` is an
                iterator whose steps emit one PE instruction of the next
                chunk's projection between attention steps."""
                kvh = jq // 2
                def fill(n):
                    for _ in range(n):
                        if next(filler, None) is None:
                            break
                for qb in range(S // NB):
                    nkj = 4 * (qb + 1)
                    base = nkj - 4
                    # diagonal chunks first (c=0..3), then full chunks
                    seq = [(base + c, c) for c in range(4)] + \
                          [(kj, None) for kj in range(base)]
                    oa = [psum_oa.tile([P, NB], F32, tag="oa", name=f"oa{p}")
                          for p in range(2)]
                    Es = {}
                    for step, (kj, c) in enumerate(seq):
                        off = P * c if c else 0
                        w = NB - off
                        for p in range(2):
                            hsl = slice(64 * p, 64 * p + 64)
                            sps = psum_sc.tile([P, NB], F32, tag="sc")
                            nc.tensor.matmul(
                                sps[:, 0:w],
                                kdup[kvh][hsl, kj * P:(kj + 1) * P],
                                qr[hsl, qb * NB + off:(qb + 1) * NB],
                                start=True, stop=True)
                            E = epool.tile([P, NB], BF, tag="E")
                            nc.scalar.activation(E[:, 0:w], sps[:, 0:w], Exp)
                            if c is not None:
                                nc.gpsimd.tensor_mul(
                                    E[:, 0:P], E[:, 0:P], maskt[:])
                            Es[p] = (E, off, w)
                        fill(2)
                        for p in range(2):
                            E, off, w = Es[p]
                            nc.tensor.matmul(
                                oa[p][:, off:NB], vaug[kvh][kj][:], E[:, 0:w],
                                start=(step == 0), stop=(step == len(seq) - 1))
                        fill(1)
                    qsl = slice(qb * NB, (qb + 1) * NB)
                    for p in range(2):
                        rec = rpool.tile([64, NB], F32, tag="rec")
                        nc.vector.reciprocal(rec[:], oa[p][64:128, :])
                        nc.vector.tensor_mul(
                            aot[jq][64 * p:64 * p + 64, qsl],
                            oa[p][0:64, :], rec[:])

            # ---- pre-attention: K, V, and the first q chunk ----
            emit_k(0, 0)
            emit_k(1, 0)
            emit_k(0, 1)
            emit_k(1, 1)
            for ic in range(S // P):
                emit_v(ic)
            qr0 = qrtp.tile([P, S], BF, tag="qr")
            for _ in proj_q_steps(0, wq_pair0, qr0):
                pass

            # ---- attention over q chunks, next-chunk projection drip-fed
            wo_pairs = {}

            def load_wo_pair(pair):
                wog = wop.tile([P, 8 * 256], BF, tag="wopair")
                nc.sync.dma_start(
                    wog[:], woH[:, pair * 8 * 256:(pair + 1) * 8 * 256])
                return wog

            wq_tiles = {0: wq_pair0}
            cur_qr = qr0
            for jq in range(8):
                nxt = jq + 1
                if nxt < 8:
                    pr = nxt // 2
                    # prefetch the following pair one attention block early
                    if nxt % 2 == 1 and pr + 1 < 4 and pr + 1 not in wq_tiles:
                        wq_tiles[pr + 1] = load_wq_pair(pr + 1)
                    nxt_qr = qrtp.tile([P, S], BF, tag="qr")
                    filler = proj_q_steps(nxt, wq_tiles[pr], nxt_qr)
                else:
                    filler = iter(())
                if jq == 5:
                    wo_pairs[0] = load_wo_pair(0)
                    wo_pairs[1] = load_wo_pair(1)
                emit_attention(jq, cur_qr, filler)
                for _ in filler:     # drain any leftover projection steps
                    pass
                if nxt < 8:
                    cur_qr = nxt_qr

            # ---- output projection ----
            for n in range(DIM // P):
                pair = n // 2
                if n % 2 == 0 and pair + 1 < 8 and pair + 1 not in wo_pairs:
                    wo_pairs[pair + 1] = load_wo_pair(pair + 1)
                wog = wo_pairs[pair]
                off = (n % 2) * P
                for ib in range(2):
                    fps = psum_mm.tile([P, NB], F32, tag="mm")
                    for hd in range(8):
                        nc.tensor.matmul(
                            fps[:], wog[:, hd * 256 + off:hd * 256 + off + P],
                            aot[hd][:, ib * NB:(ib + 1) * NB],
                            start=(hd == 0), stop=(hd == 7))
                    osb = outp.tile([P, NB], F32, tag="osb")
                    if (n + ib) % 2 == 0:
                        nc.scalar.copy(osb[:], fps[:])
                    else:
                        nc.vector.tensor_copy(osb[:], fps[:])
                    nc.sync.dma_start(
                        outT[n * P:(n + 1) * P, ib * NB:(ib + 1) * NB],
                        osb[:])

    nc.compile()
    return nc


def host_inputs(x, freqs_cos, freqs_sin, wq, wk, wv, wo):
    """Build the 8 per-core input maps, pre-packed into SBUF tile layout."""
    import ml_dtypes
    bf16 = ml_dtypes.bfloat16

    x = np.asarray(x, np.float32)
    cos = np.asarray(freqs_cos, np.float32)
    sin = np.asarray(freqs_sin, np.float32)
    wq = np.asarray(wq, np.float32)
    wk = np.asarray(wk, np.float32)
    wv = np.asarray(wv, np.float32)
    wo = np.asarray(wo, np.float32)

    # de-interleave head dims: [t0 of 32 pairs | t1 of 32 pairs] per head,
    # so the rope pair-swap is a 32-partition half-swap per 64-row head.
    perm = np.concatenate([np.arange(0, D, 2), np.arange(1, D, 2)])

    # cos/sin tiles matching that row layout, [128, S] (two 64-row heads)
    cc = cos.T  # [32 pairs, S]
    ss = sin.T
    cos64 = np.concatenate([cc, cc], 0)
    sin64 = np.concatenate([-ss, ss], 0)
    cosPa = np.ascontiguousarray(
        np.concatenate([cos64, cos64], 0)).astype(bf16)
    sinPa = np.ascontiguousarray(
        np.concatenate([sin64, sin64], 0)).astype(bf16)

    # lower-triangle [128,128] mask (key j visible to query i iff j <= i)
    j = np.arange(P)[:, None]
    i = np.arange(P)[None, :]
    maskPa = np.ascontiguousarray((j <= i).astype(np.float32)).astype(bf16)

    scale = np.float32(1.0 / np.sqrt(D))
    in_maps = []
    for core in range(8):
        b, g = core // 2, core % 2

        # x: [DIM, S] -> [p, ib, c(16), e(512)]  (chunk-of-4 grouping is a
        # view detail on the SBUF side; DRAM layout is c-major per half)
        xT = x[b].T  # [2048, 1024]
        xHa = xT.reshape(KC, P, 2, NB).transpose(1, 2, 0, 3).reshape(P, -1)

        wq_g = wq[:, g * QCOLS:(g + 1) * QCOLS].reshape(DIM, HL, D)
        wq_g = (wq_g[:, :, perm] * scale).reshape(DIM, QCOLS)
        # wq: [DIM, 1024] -> [p, pair(4), c(16), e(256)]
        wqHa = wq_g.reshape(KC, P, 4, 256).transpose(1, 2, 0, 3).reshape(P, -1)

        wk_g = wk[:, g * KCOLS:(g + 1) * KCOLS].reshape(DIM, KVL, D)
        wk_g = wk_g[:, :, perm].reshape(DIM, KCOLS)
        # wk: [DIM, 256] -> [p, jk(2), c(16), e(128)]
        wkHa = wk_g.reshape(KC, P, 2, P).transpose(1, 2, 0, 3).reshape(P, -1)

        wv_g = wv[:, g * KCOLS:(g + 1) * KCOLS]
        # wv: [DIM, 256] -> [p, c(16), e(256)]
        wvHa = wv_g.reshape(KC, P, KCOLS).transpose(1, 0, 2).reshape(P, -1)

        wo_g = wo[g * QCOLS:(g + 1) * QCOLS, :]
        # wo: [1024, 2048] -> [p, pair(8), hd(8), e(256)]
        woHa = wo_g.reshape(8, P, 8, 256).transpose(1, 2, 0, 3).reshape(P, -1)

        in_maps.append({
            "xH": np.ascontiguousarray(xHa).astype(bf16),
            "wqH": np.ascontiguousarray(wqHa).astype(bf16),
            "wkH": np.ascontiguousarray(wkHa).astype(bf16),
            "wvH": np.ascontiguousarray(wvHa).astype(bf16),
            "woH": np.ascontiguousarray(woHa).astype(bf16),
            "cosP": cosPa,
            "sinP": sinPa,
            "maskP": maskPa,
        })
    return in_maps


_PROGRAM = None


def kernel(x, freqs_cos, freqs_sin, wq, wk, wv, wo):
    global _PROGRAM
    if _PROGRAM is None:
        _PROGRAM = build_program()
    nc = _PROGRAM
    in_maps = host_inputs(x, freqs_cos, freqs_sin, wq, wk, wv, wo)
    trace = os.environ.get("KERNEL_TRACE", "") == "1"
    if not trace:
        # the axon build here lacks the NTFF profile hook; make sure an
        # ambient BASS_TRACE can't route us into that (crashing) path
        os.environ["BASS_NEVER_TRACE"] = "1"
    res = run_bass_kernel_spmd(nc, in_maps, core_ids=list(range(8)),
                               trace=trace)
    if trace and res.exec_time_ns is not None:
        print(f"HW exec time: {res.exec_time_ns} ns")
        print(f"mean exec time: {res.mean_exec_time_ns} ns")
        if res.instructions_and_trace is not None:
            print("trace:", res.instructions_and_trace[1])
    out = np.zeros((B, S, DIM), np.float32)
    for core in range(8):
        b = core // 2
        out[b] += res.results[core]["outT"].T
    return out


# revision 18
# speedup vs baseline: 1.0599x; 1.0599x over previous
"""Causal GQA attention with RoPE for Trainium2, sharded over 8 NeuronCores.

Problem: x[4,1024,2048] @ wq/wk/wv -> RoPE -> causal GQA attention -> @ wo.
H=32 q-heads, KVH=8 kv-heads (GQA rep 4), D=64.

Sharding: core = 2*b + g  (b = batch 0..3, g = head-group 0..1).
Each core handles one batch and 16 q-heads / 4 kv-heads, computing a partial
output projection; the host sums the two head-group partials per batch.

v2 design notes (all timings per the TRN2 cost model):
  - every matmul operand is bfloat16: full PE rate at any moving width
    (fp32r drops to 1/4 rate below 256 cols), half the DMA traffic, and
    2x DVE throughput on 16-bit elementwise ops.  PSUM accumulation stays
    fp32; softmax denominators and the output stay fp32.
  - all inputs are pre-packed on the host into the exact SBUF tile layout,
    so every DMA is a plain contiguous [128, N] copy with >=512B runs
    (full DMA rate) and the arrival order can be scheduled precisely.
  - rope: the pair-swap+multiply is fused into scalar_tensor_tensor ops
    ((psum * 1.0) * cos/sin) on the Pool engine (idle otherwise, no
    SBUF-access bubble); only the final add runs on DVE.  The Activation
    engine is left almost exclusively to softmax exp, which is the
    attention-phase rate limiter.
  - softmax row-sums come from 64 ones-columns appended to v: the attn@v
    matmul then lands sum(E) replicated on psum partitions 64..127, so
    normalization is one DVE reciprocal + one Pool multiply (no gpsimd
    partition_broadcast on the critical path).
  - causal masking at 128-key granularity (36 of 64 blocks per head, the
    optimum); diagonal blocks are emitted FIRST within each accumulation
    so their extra exp+mask latency hides behind the full blocks.
  - attention for the two heads of a q-chunk is interleaved kj-step by
    kj-step, and the next q-chunk's projection chain is drip-fed between
    steps, keeping the PE stream dense while exp latency drains.
"""

import os

import numpy as np

import concourse.bacc as bacc
import concourse.bass as bass
import concourse.mybir as mybir
import concourse.tile as tile
from concourse.bass_utils import run_bass_kernel_spmd

B, S, DIM = 4, 1024, 2048
H, KVH, D = 32, 8, 64
HL = H // 2        # 16 q heads per core
KVL = KVH // 2     # 4 kv heads per core
QCOLS = HL * D     # 1024
KCOLS = KVL * D    # 256
NB = 512           # matmul moving-dim block (one PSUM bank of fp32)
P = 128
KC = DIM // P      # 16 contraction chunks

F32 = mybir.dt.float32
BF = mybir.dt.bfloat16
Exp = mybir.ActivationFunctionType.Exp
MULT = mybir.AluOpType.mult


def build_program():
    nc = bacc.Bacc()

    # host-prepacked inputs: each is already in SBUF tile layout, so DMAs
    # are contiguous [128, N] row copies at full DMA rate.
    xH = nc.dram_tensor("xH", [P, 2 * KC * NB], BF, kind="ExternalInput")
    wkH = nc.dram_tensor("wkH", [P, 2 * KC * P], BF, kind="ExternalInput")
    wqH = nc.dram_tensor("wqH", [P, 4 * KC * 256], BF, kind="ExternalInput")
    wvH = nc.dram_tensor("wvH", [P, KC * KCOLS], BF, kind="ExternalInput")
    woH = nc.dram_tensor("woH", [P, 8 * 8 * 256], BF, kind="ExternalInput")
    cosP = nc.dram_tensor("cosP", [P, S], BF, kind="ExternalInput")
    sinP = nc.dram_tensor("sinP", [P, S], BF, kind="ExternalInput")
    maskP = nc.dram_tensor("maskP", [P, P], BF, kind="ExternalInput")
    outT = nc.dram_tensor("outT", [DIM, S], F32, kind="ExternalOutput")

    with tile.TileContext(nc) as tc:
        from contextlib import ExitStack
        es = ExitStack()
        with es:
            const = es.enter_context(tc.tile_pool(name="const", bufs=1))
            xtp = es.enter_context(tc.tile_pool(name="xtp", bufs=1))
            wkp = es.enter_context(tc.tile_pool(name="wkp", bufs=1))
            wvrp = es.enter_context(tc.tile_pool(name="wvrp", bufs=1))
            wstp = es.enter_context(tc.tile_pool(name="wstp", bufs=3))
            wop = es.enter_context(tc.tile_pool(name="wop", bufs=3))
            kdupp = es.enter_context(tc.tile_pool(name="kdupp", bufs=1))
            vaugp = es.enter_context(tc.tile_pool(name="vaugp", bufs=1))
            aotp = es.enter_context(tc.tile_pool(name="aotp", bufs=1))
            qrtp = es.enter_context(tc.tile_pool(name="qrtp", bufs=3))
            spool = es.enter_context(tc.tile_pool(name="spool", bufs=3))
            epool = es.enter_context(tc.tile_pool(name="epool", bufs=4))
            rpool = es.enter_context(tc.tile_pool(name="rpool", bufs=2))
            outp = es.enter_context(tc.tile_pool(name="outp", bufs=3))
            psum_mm = es.enter_context(
                tc.tile_pool(name="psum_mm", bufs=3, space="PSUM"))
            psum_oa = es.enter_context(
                tc.tile_pool(name="psum_oa", bufs=2, space="PSUM"))
            psum_sc = es.enter_context(
                tc.tile_pool(name="psum_sc", bufs=3, space="PSUM"))

            # ---- persistent tiles ----
            cost = const.tile([P, S], BF, name="cost")
            sint = const.tile([P, S], BF, name="sint")
            maskt = const.tile([P, P], BF, name="maskt")
            kdup = [kdupp.tile([P, S], BF, name=f"kdup{i}") for i in range(KVL)]
            # v with 64 ones-columns: attn@v then produces the softmax
            # denominator replicated on psum partitions 64..127.
            vaug = [[vaugp.tile([P, D + 64], BF, name=f"vaug{kv}_{ic}")
                     for ic in range(S // P)] for kv in range(KVL)]
            aot = [aotp.tile([P, S], BF, name=f"aot{j}") for j in range(8)]

            # x tiles: xt[ib][g] holds chunks 4g..4g+3, columns ib*512..+512
            xt_half = [[xtp.tile([P, 4 * NB], BF, name=f"xt{ib}_{g}")
                        for g in range(4)] for ib in range(2)]

            def xchunk(c, ib):       # [P, NB] view of x chunk c, col block ib
                g, cc = c // 4, c % 4
                return xt_half[ib][g][:, cc * NB:(cc + 1) * NB]

            wkg = [wkp.tile([P, KC * P], BF, name=f"wkg{jk}")
                   for jk in range(2)]
            wvall = wvrp.tile([P, KC * KCOLS], BF, name="wvall")
            wvt = [wvall[:, c * KCOLS:(c + 1) * KCOLS] for c in range(KC)]

            # ---- DMA issue order (single serial queue; first-needed first)
            nc.sync.dma_start(xt_half[0][0][:], xH[:, 0:4 * NB])
            nc.sync.dma_start(wkg[0][:], wkH[:, 0:KC * P])
            nc.sync.dma_start(cost[:], cosP[:])
            nc.sync.dma_start(sint[:], sinP[:])
            for g in range(1, 4):
                nc.sync.dma_start(xt_half[0][g][:],
                                  xH[:, g * 4 * NB:(g + 1) * 4 * NB])
            nc.sync.dma_start(wkg[1][:], wkH[:, KC * P:2 * KC * P])
            base1 = 4 * NB * 4
            for g in range(4):
                nc.sync.dma_start(
                    xt_half[1][g][:],
                    xH[:, base1 + g * 4 * NB:base1 + (g + 1) * 4 * NB])
            nc.sync.dma_start(wvall[:], wvH[:])

            def load_wq_pair(pair):
                wqg = wstp.tile([P, KC * 256], BF, tag="wqpair")
                nc.sync.dma_start(
                    wqg[:], wqH[:, pair * KC * 256:(pair + 1) * KC * 256])
                return wqg

            wq_pair0 = load_wq_pair(0)
            nc.sync.dma_start(maskt[:], maskP[:])

            # ones-columns of vaug (constant, disjoint from the v writes)
            for kv in range(KVL):
                for ic in range(S // P):
                    nc.gpsimd.memset(vaug[kv][ic][:, D:D + 64], 1.0)

            # ---- helpers ----
            # dims are de-interleaved so each 32-partition quadrant holds
            # [t0 of 16 pairs | t1 of the same 16 pairs]: the rope pair-swap
            # is then ONE quadrant-local stream_shuffle on DVE.
            SWAP_MASK = list(range(16, 32)) + list(range(16))

            def rope(ps, ib, dest_ap):
                """psum [128, NB] fp32 -> roped bf16 into dest_ap."""
                sl = slice(ib * NB, (ib + 1) * NB)
                straight = spool.tile([P, NB], BF, tag="straight")
                nc.scalar.copy(straight[:], ps[:])
                swapt = spool.tile([P, NB], BF, tag="swapt")
                nc.vector.stream_shuffle(swapt[:], straight[:], SWAP_MASK)
                nc.vector.tensor_mul(straight[:], straight[:], cost[:, sl])
                nc.vector.tensor_mul(swapt[:], swapt[:], sint[:, sl])
                nc.vector.tensor_add(dest_ap, straight[:], swapt[:])

            def emit_k(jk, ib):
                ps = psum_mm.tile([P, NB], F32, tag="mm")
                for c in range(KC):
                    nc.tensor.matmul(
                        ps[:], wkg[jk][:, c * P:(c + 1) * P], xchunk(c, ib),
                        start=(c == 0), stop=(c == KC - 1))
                kr = spool.tile([P, NB], BF, tag="ropek")
                rope(ps, ib, kr[:])
                sl = slice(ib * NB, (ib + 1) * NB)
                for half in range(2):     # kv head 2jk+half, duplicated
                    src = kr[64 * half:64 * half + 64, :]
                    nc.gpsimd.tensor_copy(kdup[2 * jk + half][0:64, sl], src)
                    nc.gpsimd.tensor_copy(kdup[2 * jk + half][64:128, sl], src)

            def emit_v(ic):
                ps = psum_mm.tile([P, KCOLS], F32, tag="mm")
                ib, icc = ic // 4, ic % 4
                for c in range(KC):
                    nc.tensor.matmul(
                        ps[:], xchunk(c, ib)[:, icc * P:(icc + 1) * P],
                        wvt[c][:], start=(c == 0), stop=(c == KC - 1))
                for kv in range(KVL):
                    nc.scalar.copy(vaug[kv][ic][:, 0:D],
                                   ps[:, kv * D:(kv + 1) * D])

            def proj_q_steps(jq, wqg, qr):
                """Yield per-matmul steps of the q projection for chunk jq
                (both 512-col blocks), with the rope chains emitted after
                each block completes."""
                off = (jq % 2) * P
                for ib in range(2):
                    ps = psum_mm.tile([P, NB], F32, tag="mm")
                    for c in range(KC):
                        nc.tensor.matmul(
                            ps[:], wqg[:, c * 256 + off:c * 256 + off + P],
                            xchunk(c, ib), start=(c == 0), stop=(c == KC - 1))
                        yield
                    rope(ps, ib, qr[:, ib * NB:(ib + 1) * NB])
                    yield

            def emit_attention(jq, qr, filler):
                """Attention for the two heads of q-chunk jq; `filler` is an
                iterator whose steps emit one PE instruction of the next
                chunk's projection between attention steps."""
                kvh = jq // 2
                def fill(n):
                    for _ in range(n):
                        if next(filler, None) is None:
                            break
                for qb in range(S // NB):
                    nkj = 4 * (qb + 1)
                    base = nkj - 4
                    # diagonal chunks first (c=0..3), then full chunks
                    seq = [(base + c, c) for c in range(4)] + \
                          [(kj, None) for kj in range(base)]
                    oa = [psum_oa.tile([P, NB], F32, tag="oa", name=f"oa{p}")
                          for p in range(2)]
                    Es = {}
                    for step, (kj, c) in enumerate(seq):
                        off = P * c if c else 0
                        w = NB - off
                        for p in range(2):
                            hsl = slice(64 * p, 64 * p + 64)
                            sps = psum_sc.tile([P, NB], F32, tag="sc")
                            nc.tensor.matmul(
                                sps[:, 0:w],
                                kdup[kvh][hsl, kj * P:(kj + 1) * P],
                                qr[hsl, qb * NB + off:(qb + 1) * NB],
                                start=True, stop=True)
                            E = epool.tile([P, NB], BF, tag="E")
                            nc.scalar.activation(E[:, 0:w], sps[:, 0:w], Exp)
                            if c is not None:
                                nc.gpsimd.tensor_mul(
                                    E[:, 0:P], E[:, 0:P], maskt[:])
                            Es[p] = (E, off, w)
                        fill(2)
                        for p in range(2):
                            E, off, w = Es[p]
                            nc.tensor.matmul(
                                oa[p][:, off:NB], vaug[kvh][kj][:], E[:, 0:w],
                                start=(step == 0), stop=(step == len(seq) - 1))
                        fill(1)
                    qsl = slice(qb * NB, (qb + 1) * NB)
                    for p in range(2):
                        rec = rpool.tile([64, NB], F32, tag="rec")
                        nc.vector.reciprocal(rec[:], oa[p][64:128, :])
                        nc.vector.tensor_mul(
                            aot[jq][64 * p:64 * p + 64, qsl],
                            oa[p][0:64, :], rec[:])

            # ---- pre-attention: K, V, and the first q chunk ----
            emit_k(0, 0)
            emit_k(1, 0)
            emit_k(0, 1)
            emit_k(1, 1)
            for ic in range(S // P):
                emit_v(ic)
            qr0 = qrtp.tile([P, S], BF, tag="qr")
            for _ in proj_q_steps(0, wq_pair0, qr0):
                pass

            # ---- attention over q chunks, next-chunk projection drip-fed
            wo_pairs = {}

            def load_wo_pair(pair):
                wog = wop.tile([P, 8 * 256], BF, tag="wopair")
                nc.sync.dma_start(
                    wog[:], woH[:, pair * 8 * 256:(pair + 1) * 8 * 256])
                return wog

            wq_tiles = {0: wq_pair0}
            cur_qr = qr0
            for jq in range(8):
                nxt = jq + 1
                if nxt < 8:
                    pr = nxt // 2
                    # prefetch the following pair one attention block early
                    if nxt % 2 == 1 and pr + 1 < 4 and pr + 1 not in wq_tiles:
                        wq_tiles[pr + 1] = load_wq_pair(pr + 1)
                    nxt_qr = qrtp.tile([P, S], BF, tag="qr")
                    filler = proj_q_steps(nxt, wq_tiles[pr], nxt_qr)
                else:
                    filler = iter(())
                if jq == 5:
                    wo_pairs[0] = load_wo_pair(0)
                    wo_pairs[1] = load_wo_pair(1)
                emit_attention(jq, cur_qr, filler)
                for _ in filler:     # drain any leftover projection steps
                    pass
                if nxt < 8:
                    cur_qr = nxt_qr

            # ---- output projection ----
            for n in range(DIM // P):
                pair = n // 2
                if n % 2 == 0 and pair + 1 < 8 and pair + 1 not in wo_pairs:
                    wo_pairs[pair + 1] = load_wo_pair(pair + 1)
                wog = wo_pairs[pair]
                off = (n % 2) * P
                for ib in range(2):
                    fps = psum_mm.tile([P, NB], F32, tag="mm")
                    for hd in range(8):
                        nc.tensor.matmul(
                            fps[:], wog[:, hd * 256 + off:hd * 256 + off + P],
                            aot[hd][:, ib * NB:(ib + 1) * NB],
                            start=(hd == 0), stop=(hd == 7))
                    osb = outp.tile([P, NB], F32, tag="osb")
                    if (n + ib) % 2 == 0:
                        nc.scalar.copy(osb[:], fps[:])
                    else:
                        nc.vector.tensor_copy(osb[:], fps[:])
                    nc.sync.dma_start(
                        outT[n * P:(n + 1) * P, ib * NB:(ib + 1) * NB],
                        osb[:])

    nc.compile()
    return nc


def host_inputs(x, freqs_cos, freqs_sin, wq, wk, wv, wo):
    """Build the 8 per-core input maps, pre-packed into SBUF tile layout."""
    import ml_dtypes
    bf16 = ml_dtypes.bfloat16

    x = np.asarray(x, np.float32)
    cos = np.asarray(freqs_cos, np.float32)
    sin = np.asarray(freqs_sin, np.float32)
    wq = np.asarray(wq, np.float32)
    wk = np.asarray(wk, np.float32)
    wv = np.asarray(wv, np.float32)
    wo = np.asarray(wo, np.float32)

    # de-interleave per 32-partition quadrant: each quadrant holds
    # [t0 of 16 pairs | t1 of the same 16 pairs] so the rope pair-swap is
    # a quadrant-local 16<->16 stream_shuffle.
    perm = np.concatenate([np.arange(0, 32, 2), np.arange(1, 32, 2),
                           np.arange(32, 64, 2), np.arange(33, 64, 2)])

    # cos/sin tiles matching that row layout, [128, S] (two 64-row heads)
    cc = cos.T  # [32 pairs, S]
    ss = sin.T
    cos64 = np.concatenate([cc[0:16], cc[0:16], cc[16:32], cc[16:32]], 0)
    sin64 = np.concatenate([-ss[0:16], ss[0:16], -ss[16:32], ss[16:32]], 0)
    cosPa = np.ascontiguousarray(
        np.concatenate([cos64, cos64], 0)).astype(bf16)
    sinPa = np.ascontiguousarray(
        np.concatenate([sin64, sin64], 0)).astype(bf16)

    # lower-triangle [128,128] mask (key j visible to query i iff j <= i)
    j = np.arange(P)[:, None]
    i = np.arange(P)[None, :]
    maskPa = np.ascontiguousarray((j <= i).astype(np.float32)).astype(bf16)

    scale = np.float32(1.0 / np.sqrt(D))
    in_maps = []
    for core in range(8):
        b, g = core // 2, core % 2

        # x: [DIM, S] -> [p, ib, c(16), e(512)]  (chunk-of-4 grouping is a
        # view detail on the SBUF side; DRAM layout is c-major per half)
        xT = x[b].T  # [2048, 1024]
        xHa = xT.reshape(KC, P, 2, NB).transpose(1, 2, 0, 3).reshape(P, -1)

        wq_g = wq[:, g * QCOLS:(g + 1) * QCOLS].reshape(DIM, HL, D)
        wq_g = (wq_g[:, :, perm] * scale).reshape(DIM, QCOLS)
        # wq: [DIM, 1024] -> [p, pair(4), c(16), e(256)]
        wqHa = wq_g.reshape(KC, P, 4, 256).transpose(1, 2, 0, 3).reshape(P, -1)

        wk_g = wk[:, g * KCOLS:(g + 1) * KCOLS].reshape(DIM, KVL, D)
        wk_g = wk_g[:, :, perm].reshape(DIM, KCOLS)
        # wk: [DIM, 256] -> [p, jk(2), c(16), e(128)]
        wkHa = wk_g.reshape(KC, P, 2, P).transpose(1, 2, 0, 3).reshape(P, -1)

        wv_g = wv[:, g * KCOLS:(g + 1) * KCOLS]
        # wv: [DIM, 256] -> [p, c(16), e(256)]
        wvHa = wv_g.reshape(KC, P, KCOLS).transpose(1, 0, 2).reshape(P, -1)

        wo_g = wo[g * QCOLS:(g + 1) * QCOLS, :]
        # wo: [1024, 2048] -> [p, pair(8), hd(8), e(256)]
        woHa = wo_g.reshape(8, P, 8, 256).transpose(1, 2, 0, 3).reshape(P, -1)

        in_maps.append({
            "xH": np.ascontiguousarray(xHa).astype(bf16),
            "wqH": np.ascontiguousarray(wqHa).astype(bf16),
            "wkH": np.ascontiguousarray(wkHa).astype(bf16),
            "wvH": np.ascontiguousarray(wvHa).astype(bf16),
            "woH": np.ascontiguousarray(woHa).astype(bf16),
            "cosP": cosPa,
            "sinP": sinPa,
            "maskP": maskPa,
        })
    return in_maps


_PROGRAM = None


def kernel(x, freqs_cos, freqs_sin, wq, wk, wv, wo):
    global _PROGRAM
    if _PROGRAM is None:
        _PROGRAM = build_program()
    nc = _PROGRAM
    in_maps = host_inputs(x, freqs_cos, freqs_sin, wq, wk, wv, wo)
    trace = os.environ.get("KERNEL_TRACE", "") == "1"
    if not trace:
        # the axon build here lacks the NTFF profile hook; make sure an
        # ambient BASS_TRACE can't route us into that (crashing) path
        os.environ["BASS_NEVER_TRACE"] = "1"
    res = run_bass_kernel_spmd(nc, in_maps, core_ids=list(range(8)),
                               trace=trace)
    if trace and res.exec_time_ns is not None:
        print(f"HW exec time: {res.exec_time_ns} ns")
        print(f"mean exec time: {res.mean_exec_time_ns} ns")
        if res.instructions_and_trace is not None:
            print("trace:", res.instructions_and_trace[1])
    out = np.zeros((B, S, DIM), np.float32)
    for core in range(8):
        b = core // 2
        out[b] += res.results[core]["outT"].T
    return out
